# revision 20
# baseline (speedup 1.0000x reference)
"""Trainium2 Bass kernel for a 2-layer tanh RNN over ragged right-padded
sequences (B=64, S=512, V=32000, E=H=512, L=2), data-parallel over batch
across 8 NeuronCores (8 sequences per core).

Design (per core, all trace-time specialized):
  - Embedding gather via indirect DMA (rows of emb for this core's tokens,
    token order (t, b)).
  - X tiles PE-transposed to X^T [E, ntok] (bf16).
  - pre_l^T = W_ih[l] @ X_l^T + (b_ih+b_hh)  as big weight-stationary
    matmuls -> [H, ntok] f32 in SBUF.
  - Recurrence kept fully in transposed space: hT [H(4x128 part), 8].
    Per step, 16 (LDW+MM) pairs with bf16 stationary W_hh^T tiles (FWL),
    N=8 moving; per m-tile: DVE add of pre column, ACT tanh -> H^T store.
  - No masking: lengths only decide which column of the stored H^T
    trajectory is each sequence's final state; selection happens on host.
"""

import numpy as np
import ml_dtypes

B, S, V, E, H, L = 64, 512, 32000, 512, 512, 2
NC = 8          # cores
BL = B // NC    # sequences per core = 8
T = S           # steps (uniform across cores; SPMD single program)
NTOK = T * BL   # 4096 tokens per core
P = 128
KT = H // P     # 4 k/m tiles

BF16 = ml_dtypes.bfloat16


def _build(nc, mybir, bass, tile):
    f32 = mybir.dt.float32
    bf16 = mybir.dt.bfloat16
    i32 = mybir.dt.int32
    Tanh = mybir.ActivationFunctionType.Tanh
    Ident = mybir.ActivationFunctionType.Identity

    # ---- DRAM parameters (per-core shards prepared on host) ----
    emb_d = nc.dram_tensor("emb", [V, E], f32, kind="ExternalInput")
    xidx_d = nc.dram_tensor("xidx", [NTOK], i32, kind="ExternalInput")
    whhT_d = nc.dram_tensor("whhT", [L, H, H], bf16, kind="ExternalInput")  # W_hh[l].T
    wihT_d = nc.dram_tensor("wihT", [L, H, H], bf16, kind="ExternalInput")  # W_ih[l].T
    bsum_d = nc.dram_tensor("bsum", [L, KT, P, 1], f32, kind="ExternalInput")
    iden_d = nc.dram_tensor("iden", [P, P], f32, kind="ExternalInput")
    idenb_d = nc.dram_tensor("idenb", [P, P], bf16, kind="ExternalInput")
    hout_d = nc.dram_tensor("hout", [L, KT, P, NTOK], bf16, kind="ExternalOutput")

    NCH = NTOK // 512  # n-chunks for the pre matmuls

    with tile.TileContext(nc) as tc:
        with (
            tc.tile_pool(name="const", bufs=1) as cpool,
            tc.tile_pool(name="xt", bufs=1) as xtpool,
            tc.tile_pool(name="pre", bufs=1) as prepool,
            tc.tile_pool(name="hs", bufs=1) as hpool,
            tc.tile_pool(name="gat", bufs=1) as gpool,
            tc.tile_pool(name="pst", bufs=2, space="PSUM") as pst,
            tc.tile_pool(name="psd", bufs=1, space="PSUM") as psd,
            tc.tile_pool(name="psp", bufs=2, space="PSUM") as psp,
            tc.tile_pool(name="psr", bufs=2, space="PSUM") as psr,
        ):
            # ---- constants to SBUF ----
            iden = cpool.tile([P, P], f32, tag="iden")
            nc.sync.dma_start(iden[:], iden_d[:])
            idenb = cpool.tile([P, P], bf16, tag="idenb")
            nc.sync.dma_start(idenb[:], idenb_d[:])
            whh_t = cpool.tile([P, L, KT, H], bf16, tag="whh", name="whh_t")
            wih_t = cpool.tile([P, L, KT, H], bf16, tag="wih", name="wih_t")
            for l in range(L):
                for k in range(KT):
                    nc.sync.dma_start(whh_t[:, l, k, :], whhT_d[l, k * P:(k + 1) * P, :])
                    nc.sync.dma_start(wih_t[:, l, k, :], wihT_d[l, k * P:(k + 1) * P, :])
            whh = {(l, k): whh_t[:, l, k, :] for l in range(L) for k in range(KT)}
            wih = {(l, k): wih_t[:, l, k, :] for l in range(L) for k in range(KT)}
            bias = cpool.tile([P, L * KT], f32, tag="bias")
            nc.sync.dma_start(bias[:],
                              bsum_d.rearrange("l k p o -> p (l k o)"))
            bscr = cpool.tile([P, L * KT], f32, tag="bscr")
            nc.scalar.copy(bscr[:], bias[:])  # ACT observes the bias DMA once
            idx = cpool.tile([P, NTOK // P], i32, tag="idx")
            nc.sync.dma_start(idx[:], xidx_d.rearrange("(g p) -> p g", p=P))

            # ---- gather embeddings; DVE-cast to bf16; DMA-transpose
            #      (xbar) into X^T -- keeps the PE free of transposes ----
            XTt = xtpool.tile([P, KT, NTOK], bf16, tag="sh", name="xtt")
            XT = [XTt[:, k, :] for k in range(KT)]
            NG = NTOK // P
            xgs = [gpool.tile([P, E], f32, tag=f"xg{j}", name=f"xg{j}")
                   for j in range(NG)]
            for g in range(NG):
                xg = xgs[g]
                nc.gpsimd.indirect_dma_start(
                    out=xg[:], out_offset=None, in_=emb_d[:],
                    in_offset=bass.IndirectOffsetOnAxis(ap=idx[:, g:g + 1], axis=0),
                )
                xgb = gpool.tile([P, E], bf16, tag="xgb", name="xgb", bufs=4)
                nc.vector.tensor_copy(xgb[:], xg[:])
                nc.sync.dma_start_transpose(
                    out=XTt[:, :, g * P:(g + 1) * P], in_=xgb[:])

            H0T = hpool.tile([P, KT, NTOK], bf16, tag="h0t")

            def pre_matmuls(l, rhs_slice, preT):
                # ldweights absorb the weight-DMA / DVE waits so the real
                # matmuls only wait on ACT (psum WAR / rhs producer).
                for k in range(KT):
                    nc.tensor.ldweights(wih[l, k][:, 0:P])
                for n in range(NCH):
                    for m in range(KT):
                        ps = psp.tile([P, 512], f32, tag="psp")
                        for k in range(KT):
                            nc.tensor.matmul(
                                ps[:],
                                wih[l, k][:, m * P:(m + 1) * P],
                                rhs_slice(k, n),
                                start=(k == 0), stop=(k == KT - 1),
                            )
                        nc.scalar.activation(
                            preT[:, m, n * 512:(n + 1) * 512], ps[:], Ident,
                            bias=bias[:, l * KT + m:l * KT + m + 1],
                        )

            def recurrence(l, preT, HT):
                for k in range(KT):
                    nc.tensor.ldweights(whh[l, k][:, 0:P])
                nc.tensor.ldweights(idenb[:])
                for t in range(T):
                    pv = (t - 1) * BL
                    cur = t * BL
                    ps = psr.tile([P, KT, BL], f32, tag="psr")
                    nc.tensor.matmul(ps[:], idenb[:], preT[:, :, cur:cur + BL],
                                     start=True, stop=(t == 0),
                                     skip_group_check=True)
                    for m in range(KT):
                        if t == 0:
                            break
                        for k in range(KT):
                            nc.tensor.matmul(
                                ps[:, m, :],
                                whh[l, k][:, m * P:(m + 1) * P],
                                HT[:, k, pv:pv + BL],
                                start=False, stop=(m == KT - 1 and k == KT - 1),
                                skip_group_check=True,
                            )
                    nc.scalar.activation(HT[:, :, cur:cur + BL], ps[:], Tanh)

            # ---- layer 0 ----
            preT = prepool.tile([P, KT, NTOK], bf16, tag="pre", name="pre0")
            pre_matmuls(0, lambda k, n: XT[k][:, n * 512:(n + 1) * 512], preT)
            recurrence(0, preT, H0T)
            for k in range(KT):
                for q in range(4):
                    c0, c1 = q * (NTOK // 4), (q + 1) * (NTOK // 4)
                    nc.sync.dma_start(hout_d[0, k, :, c0:c1], H0T[:, k, c0:c1])

            # ---- layer 1 ----
            preT1 = prepool.tile([P, KT, NTOK], bf16, tag="pre", name="pre1")
            pre_matmuls(1, lambda k, n: H0T[:, k, n * 512:(n + 1) * 512], preT1)
            # absorb the DVE->XT writes into ACT before H1T reuses the slot
            xscr = cpool.tile([P, 1], bf16, tag="xscr")
            nc.scalar.copy(xscr[:], XTt[:, 0, 0:1])
            H1T = xtpool.tile([P, KT, NTOK], bf16, tag="sh", name="h1t")
            recurrence(1, preT1, H1T)
            for k in range(KT):
                for q in range(4):
                    c0, c1 = q * (NTOK // 4), (q + 1) * (NTOK // 4)
                    nc.sync.dma_start(hout_d[1, k, :, c0:c1], H1T[:, k, c0:c1])

    return nc


_CACHE = {}
LAST_RESULT = None


def _get_built():
    if "nc" not in _CACHE:
        import concourse.bass as bass
        import concourse.mybir as mybir
        from concourse import tile
        from concourse.bacc import Bacc
        nc = Bacc(trn_type="TRN2")
        _build(nc, mybir, bass, tile)
        nc.compile()
        _CACHE["nc"] = nc
    return _CACHE["nc"]


def kernel(x, lengths, emb, W_ih, W_hh, b_ih, b_hh):
    from concourse.bass_utils import run_bass_kernel_spmd

    x = np.asarray(x).astype(np.int32)
    lengths = np.asarray(lengths).astype(np.int64)
    emb = np.ascontiguousarray(np.asarray(emb, dtype=np.float32))
    whhT = np.ascontiguousarray(
        np.transpose(np.asarray(W_hh, np.float32), (0, 2, 1))).astype(BF16)
    wihT = np.ascontiguousarray(
        np.transpose(np.asarray(W_ih, np.float32), (0, 2, 1))).astype(BF16)
    bsum = (np.asarray(b_ih, np.float32) + np.asarray(b_hh, np.float32))
    bsum = np.ascontiguousarray(bsum.reshape(L, KT, P, 1))
    iden = np.eye(P, dtype=np.float32)
    idenb = np.eye(P, dtype=np.float32).astype(BF16)

    nc = _get_built()
    in_maps = []
    for c in range(NC):
        xc = x[c * BL:(c + 1) * BL]               # [8, S]
        xidx = np.ascontiguousarray(xc.T.reshape(-1)).astype(np.int32)  # (t, b)
        in_maps.append({
            "emb": emb, "xidx": xidx, "whhT": whhT, "wihT": wihT,
            "bsum": bsum, "iden": iden, "idenb": idenb,
        })

    import os
    trace = bool(os.environ.get("BASS_TRACE"))
    res = run_bass_kernel_spmd(nc, in_maps, core_ids=list(range(NC)),
                               trace=trace)
    global LAST_RESULT
    LAST_RESULT = res
    results = res.results if hasattr(res, "results") else res

    out = np.zeros((B, H), np.float32)
    hidden = np.zeros((B, L, H), np.float32)
    for c in range(NC):
        hc = np.asarray(results[c]["hout"], dtype=np.float32)  # [L,KT,P,NTOK]
        for bl in range(BL):
            b = c * BL + bl
            col = (int(lengths[b]) - 1) * BL + bl
            hvec = hc[:, :, :, col].reshape(L, H)
            hidden[b] = hvec
            out[b] = hvec[1]
    return out, hidden


# revision 21
# speedup vs baseline: 1.1570x; 1.1570x over previous
"""Trainium2 Bass kernel for a 2-layer tanh RNN over ragged right-padded
sequences (B=64, S=512, V=32000, E=H=512, L=2), data-parallel over batch
across 8 NeuronCores (8 sequences per core).

Design (per core, all trace-time specialized):
  - Embedding gather via indirect DMA (rows of emb for this core's tokens,
    token order (t, b)).
  - X tiles PE-transposed to X^T [E, ntok] (bf16).
  - pre_l^T = W_ih[l] @ X_l^T + (b_ih+b_hh)  as big weight-stationary
    matmuls -> [H, ntok] f32 in SBUF.
  - Recurrence kept fully in transposed space: hT [H(4x128 part), 8].
    Per step, 16 (LDW+MM) pairs with bf16 stationary W_hh^T tiles (FWL),
    N=8 moving; per m-tile: DVE add of pre column, ACT tanh -> H^T store.
  - No masking: lengths only decide which column of the stored H^T
    trajectory is each sequence's final state; selection happens on host.
"""

import numpy as np
import ml_dtypes

B, S, V, E, H, L = 64, 512, 32000, 512, 512, 2
NC = 8          # cores
BL = B // NC    # sequences per core = 8
T = S           # steps (uniform across cores; SPMD single program)
NTOK = T * BL   # 4096 tokens per core
P = 128
KT = H // P     # 4 k/m tiles

BF16 = ml_dtypes.bfloat16


def _build(nc, mybir, bass, tile):
    f32 = mybir.dt.float32
    bf16 = mybir.dt.bfloat16
    i32 = mybir.dt.int32
    Tanh = mybir.ActivationFunctionType.Tanh
    Ident = mybir.ActivationFunctionType.Identity

    # ---- DRAM parameters (per-core shards prepared on host) ----
    emb_d = nc.dram_tensor("emb", [V, E], f32, kind="ExternalInput")
    xidx_d = nc.dram_tensor("xidx", [NTOK], i32, kind="ExternalInput")
    whhT_d = nc.dram_tensor("whhT", [L, H, H], bf16, kind="ExternalInput")  # W_hh[l].T
    wihT_d = nc.dram_tensor("wihT", [L, H, H], bf16, kind="ExternalInput")  # W_ih[l].T
    bsum_d = nc.dram_tensor("bsum", [L, KT, P, 1], f32, kind="ExternalInput")
    iden_d = nc.dram_tensor("iden", [P, P], f32, kind="ExternalInput")
    idenb_d = nc.dram_tensor("idenb", [P, P], bf16, kind="ExternalInput")
    hout_d = nc.dram_tensor("hout", [L, KT, P, NTOK], bf16, kind="ExternalOutput")

    NCH = NTOK // 512  # n-chunks for the pre matmuls

    with tile.TileContext(nc) as tc:
        with (
            tc.tile_pool(name="const", bufs=1) as cpool,
            tc.tile_pool(name="xt", bufs=1) as xtpool,
            tc.tile_pool(name="pre", bufs=1) as prepool,
            tc.tile_pool(name="hs", bufs=1) as hpool,
            tc.tile_pool(name="gat", bufs=1) as gpool,
            tc.tile_pool(name="pst", bufs=2, space="PSUM") as pst,
            tc.tile_pool(name="psd", bufs=1, space="PSUM") as psd,
            tc.tile_pool(name="psp", bufs=2, space="PSUM") as psp,
            tc.tile_pool(name="psr", bufs=2, space="PSUM") as psr,
        ):
            # ---- constants to SBUF ----
            iden = cpool.tile([P, P], f32, tag="iden")
            nc.sync.dma_start(iden[:], iden_d[:])
            idenb = cpool.tile([P, P], bf16, tag="idenb")
            nc.sync.dma_start(idenb[:], idenb_d[:])
            whh_t = cpool.tile([P, L, KT, H], bf16, tag="whh", name="whh_t")
            wih_t = cpool.tile([P, L, KT, H], bf16, tag="wih", name="wih_t")
            for l in range(L):
                for k in range(KT):
                    nc.sync.dma_start(whh_t[:, l, k, :], whhT_d[l, k * P:(k + 1) * P, :])
                    nc.sync.dma_start(wih_t[:, l, k, :], wihT_d[l, k * P:(k + 1) * P, :])
            whh = {(l, k): whh_t[:, l, k, :] for l in range(L) for k in range(KT)}
            wih = {(l, k): wih_t[:, l, k, :] for l in range(L) for k in range(KT)}
            bias = cpool.tile([P, L * KT], f32, tag="bias")
            nc.sync.dma_start(bias[:],
                              bsum_d.rearrange("l k p o -> p (l k o)"))
            bscr = cpool.tile([P, L * KT], f32, tag="bscr")
            nc.scalar.copy(bscr[:], bias[:])  # ACT observes the bias DMA once
            idx = cpool.tile([P, NTOK // P], i32, tag="idx")
            nc.sync.dma_start(idx[:], xidx_d.rearrange("(g p) -> p g", p=P))

            # ---- gather embeddings + PE-transpose to XT (bf16) ----
            # A dummy 1x1 matmul absorbs each gather's DMA-queue wait into
            # PE (instructions can carry only one sync wait), so the real
            # transposes only ever wait on the DVE (pt WAR) semaphore.
            XTt = xtpool.tile([P, KT, NTOK], bf16, tag="sh", name="xtt")
            XT = [XTt[:, k, :] for k in range(KT)]
            NG = NTOK // P
            dps = psd.tile([1, 1], f32, tag="dps")
            xgs = [gpool.tile([P, E], f32, tag=f"xg{j}", name=f"xg{j}")
                   for j in range(NG)]
            for g in range(NG):
                xg = xgs[g]
                nc.gpsimd.indirect_dma_start(
                    out=xg[:], out_offset=None, in_=emb_d[:],
                    in_offset=bass.IndirectOffsetOnAxis(ap=idx[:, g:g + 1], axis=0),
                )
                nc.tensor.matmul(dps[:], iden[:, 0:1], xg[:, 0:1],
                                 start=(g == 0), stop=(g == NG - 1),
                                 skip_group_check=True)
                for e in range(KT):
                    pt = pst.tile([P, P], f32, tag="pt")
                    nc.tensor.transpose(out=pt[:], in_=xg[:, e * P:(e + 1) * P],
                                        identity=iden[:])
                    nc.vector.tensor_copy(XT[e][:, g * P:(g + 1) * P], pt[:])

            H0T = hpool.tile([P, KT, NTOK], bf16, tag="h0t")

            def pre_matmuls(l, rhs_slice, preT):
                # ldweights absorb the weight-DMA / DVE waits so the real
                # matmuls only wait on ACT (psum WAR / rhs producer).
                for k in range(KT):
                    nc.tensor.ldweights(wih[l, k][:, 0:P])
                for n in range(NCH):
                    for m in range(KT):
                        ps = psp.tile([P, 512], f32, tag="psp")
                        for k in range(KT):
                            nc.tensor.matmul(
                                ps[:],
                                wih[l, k][:, m * P:(m + 1) * P],
                                rhs_slice(k, n),
                                start=(k == 0), stop=(k == KT - 1),
                            )
                        nc.scalar.activation(
                            preT[:, m, n * 512:(n + 1) * 512], ps[:], Ident,
                            bias=bias[:, l * KT + m:l * KT + m + 1],
                        )

            def recurrence(l, preT, HT):
                for k in range(KT):
                    nc.tensor.ldweights(whh[l, k][:, 0:P])
                nc.tensor.ldweights(idenb[:])
                for t in range(T):
                    pv = (t - 1) * BL
                    cur = t * BL
                    ps = psr.tile([P, KT, BL], f32, tag="psr")
                    nc.tensor.matmul(ps[:], idenb[:], preT[:, :, cur:cur + BL],
                                     start=True, stop=(t == 0),
                                     skip_group_check=True)
                    for m in range(KT):
                        if t == 0:
                            break
                        for k in range(KT):
                            nc.tensor.matmul(
                                ps[:, m, :],
                                whh[l, k][:, m * P:(m + 1) * P],
                                HT[:, k, pv:pv + BL],
                                start=False, stop=(m == KT - 1 and k == KT - 1),
                                skip_group_check=True,
                            )
                    nc.scalar.activation(HT[:, :, cur:cur + BL], ps[:], Tanh)

            # ---- layer 0 ----
            preT = prepool.tile([P, KT, NTOK], bf16, tag="pre", name="pre0")
            pre_matmuls(0, lambda k, n: XT[k][:, n * 512:(n + 1) * 512], preT)
            recurrence(0, preT, H0T)
            for k in range(KT):
                for q in range(4):
                    c0, c1 = q * (NTOK // 4), (q + 1) * (NTOK // 4)
                    nc.sync.dma_start(hout_d[0, k, :, c0:c1], H0T[:, k, c0:c1])

            # ---- layer 1 ----
            preT1 = prepool.tile([P, KT, NTOK], bf16, tag="pre", name="pre1")
            pre_matmuls(1, lambda k, n: H0T[:, k, n * 512:(n + 1) * 512], preT1)
            # absorb the DVE->XT writes into ACT before H1T reuses the slot
            xscr = cpool.tile([P, 1], bf16, tag="xscr")
            nc.scalar.copy(xscr[:], XTt[:, 0, 0:1])
            H1T = xtpool.tile([P, KT, NTOK], bf16, tag="sh", name="h1t")
            recurrence(1, preT1, H1T)
            for k in range(KT):
                for q in range(4):
                    c0, c1 = q * (NTOK // 4), (q + 1) * (NTOK // 4)
                    nc.sync.dma_start(hout_d[1, k, :, c0:c1], H1T[:, k, c0:c1])

    return nc


_CACHE = {}
LAST_RESULT = None


def _get_built():
    if "nc" not in _CACHE:
        import concourse.bass as bass
        import concourse.mybir as mybir
        from concourse import tile
        from concourse.bacc import Bacc
        nc = Bacc(trn_type="TRN2")
        _build(nc, mybir, bass, tile)
        nc.compile()
        _CACHE["nc"] = nc
    return _CACHE["nc"]


def kernel(x, lengths, emb, W_ih, W_hh, b_ih, b_hh):
    from concourse.bass_utils import run_bass_kernel_spmd

    x = np.asarray(x).astype(np.int32)
    lengths = np.asarray(lengths).astype(np.int64)
    emb = np.ascontiguousarray(np.asarray(emb, dtype=np.float32))
    whhT = np.ascontiguousarray(
        np.transpose(np.asarray(W_hh, np.float32), (0, 2, 1))).astype(BF16)
    wihT = np.ascontiguousarray(
        np.transpose(np.asarray(W_ih, np.float32), (0, 2, 1))).astype(BF16)
    bsum = (np.asarray(b_ih, np.float32) + np.asarray(b_hh, np.float32))
    bsum = np.ascontiguousarray(bsum.reshape(L, KT, P, 1))
    iden = np.eye(P, dtype=np.float32)
    idenb = np.eye(P, dtype=np.float32).astype(BF16)

    nc = _get_built()
    in_maps = []
    for c in range(NC):
        xc = x[c * BL:(c + 1) * BL]               # [8, S]
        xidx = np.ascontiguousarray(xc.T.reshape(-1)).astype(np.int32)  # (t, b)
        in_maps.append({
            "emb": emb, "xidx": xidx, "whhT": whhT, "wihT": wihT,
            "bsum": bsum, "iden": iden, "idenb": idenb,
        })

    import os
    trace = bool(os.environ.get("BASS_TRACE"))
    res = run_bass_kernel_spmd(nc, in_maps, core_ids=list(range(NC)),
                               trace=trace)
    global LAST_RESULT
    LAST_RESULT = res
    results = res.results if hasattr(res, "results") else res

    out = np.zeros((B, H), np.float32)
    hidden = np.zeros((B, L, H), np.float32)
    for c in range(NC):
        hc = np.asarray(results[c]["hout"], dtype=np.float32)  # [L,KT,P,NTOK]
        for bl in range(BL):
            b = c * BL + bl
            col = (int(lengths[b]) - 1) * BL + bl
            hvec = hc[:, :, :, col].reshape(L, H)
            hidden[b] = hvec
            out[b] = hvec[1]
    return out, hidden
